# revision 1
# baseline (speedup 1.0000x reference)
"""EnhancedMultiHeadAttention TRN2 kernel (8 NeuronCores).

Problem (hardcoded shapes): B=4, L=1024, HID=1024, H=16, DH=64, MAX_SEQ=1024.
  q/k/v = x @ W* + b*          (per-head split)
  S = q k^T / sqrt(64) + einsum('bhid,ijd->bhij', q, rel_emb[i-j+1023])
  attn = softmax(S); out = (attn @ v) @ Wo + bo

Sharding: core c -> batch b = c//2, head group g = c%2 (8 heads each).
Each core computes a partial output x[b]-block @ Wo-rows; host sums the two
partials per batch.

Rel-pos bias trick: qE[i, r'] = q_i . rel_emb[2046-r'] (one matmul against the
flipped table), then bias[i, j] = qE[i, 1023-i+j] is a diagonal re-read of qE
with row stride 1151 inside a [128, 1152] window - done as an SBUF->SBUF DMA
with a hand-built access pattern (flat-element semantics verified on HW).

Attention transpose for the A.V matmul: one whole-head bf16 DMA block-
transpose [128,(t,jt),jj] -> [128,t,jt,pp] (the xbar transposes each 128x128
block in place; verified on HW).

Matmuls run as float32r (full-rate at N>=512, ~1e-4 rel err); attention
probabilities and the rel-bias payload travel as bf16. Head pairs are
software-pipelined (pair p's A.V emitted after pair p+1's score phase) so the
PE is never parked behind the transpose DMA.
"""

import ml_dtypes
import numpy as np

import concourse.bass as bass
import concourse.mybir as mybir
import concourse.tile as tile
from concourse.bass_utils import run_bass_kernel_spmd

B, L, HID, H = 4, 1024, 1024, 16
DH = 64
HPC = 8          # heads per core
NPAIR = 4        # head pairs per core
NT = L // 128    # 8 i-tiles
F32R = mybir.dt.float32r
F32 = mybir.dt.float32
BF16 = mybir.dt.bfloat16

_uid = [0]


def _split_multi_waits(nc):
    """Installed walrus accepts 1 sync-wait per instruction (2 for
    EventSemaphore); Tile's tail drain can carry more. Spill extras onto
    EventSemaphore wait-carriers inserted before the offender."""
    for f in nc.m.functions:
        for blk in f.blocks:
            insts = blk.instructions
            idx = 0
            while idx < len(insts):
                inst = insts[idx]
                si = inst.sync_info
                waits = list(si.on_wait) if si and si.on_wait else []
                cap = 2 if type(inst).__name__ == "InstEventSemaphore" else 1
                if len(waits) > cap:
                    si.on_wait = waits[:cap]
                    extra = waits[cap:]
                    carriers = []
                    for k in range(0, len(extra), 2):
                        _uid[0] += 1
                        nop = mybir.InstEventSemaphore(
                            name=f"wait_split_{_uid[0]}", ins=[], outs=[]
                        )
                        nop.engine = inst.engine
                        nop.sync_info = mybir.SyncInfo(
                            on_wait=extra[k:k + 2], on_update=[]
                        )
                        carriers.append(nop)
                    for c in reversed(carriers):
                        insts.insert(idx, c)
                    idx += len(carriers)
                idx += 1


def _ap_with(ap, dims, offset):
    """Return a copy of `ap` with raw [step,count] dims and element offset."""
    c = ap.copy()
    v = c.ap
    assert len(v) == len(dims), (v, dims)
    for i, d in enumerate(dims):
        v[i] = list(d)
    c.ap = v
    c.offset = offset
    return c


def _build_program():
    nc = bass.Bass()

    xT = nc.dram_tensor("xT", (HID, L), F32R, kind="ExternalInput")
    wq = nc.dram_tensor("wq", (HID, 512), F32R, kind="ExternalInput")
    wk = nc.dram_tensor("wk", (HID, 512), F32R, kind="ExternalInput")
    wv = nc.dram_tensor("wv", (HID, 512), F32R, kind="ExternalInput")
    wo = nc.dram_tensor("wo", (512, L), F32R, kind="ExternalInput")
    rt = nc.dram_tensor("rt", (128, 2048), BF16, kind="ExternalInput")
    bq = nc.dram_tensor("bq", (512,), F32, kind="ExternalInput")
    bk = nc.dram_tensor("bk", (512,), F32, kind="ExternalInput")
    bv = nc.dram_tensor("bv", (512,), F32, kind="ExternalInput")
    bo = nc.dram_tensor("bo", (L,), F32, kind="ExternalInput")
    out = nc.dram_tensor("out", (L, L), F32, kind="ExternalOutput")

    with tile.TileContext(nc) as tc:
        with tc.tile_pool(name="weights", bufs=1) as wpool, \
             tc.tile_pool(name="proj", bufs=1) as projpool, \
             tc.tile_pool(name="ps", bufs=8, space="PSUM") as ps:

            # ---- resident small tensors ----
            rt_sb = wpool.tile([128, 2048], BF16)
            nc.sync.dma_start(rt_sb[:], rt[:])
            bq_sb = wpool.tile([128, 4], F32)
            nc.sync.dma_start(bq_sb[:], bq[:].rearrange("(t p) -> p t", p=128))
            bk_sb = wpool.tile([128, 4], F32)
            nc.sync.dma_start(bk_sb[:], bk[:].rearrange("(t p) -> p t", p=128))
            # bv replicated across partitions: [1,512] read with partition step 0
            bv_sb = wpool.tile([128, 512], F32)
            nc.sync.dma_start(bv_sb[:], _ap_with(bv[None, :], [[0, 128], [1, 512]], 0))
            bo_sb = wpool.tile([128, 1024], F32)
            nc.sync.dma_start(bo_sb[:], _ap_with(bo[None, :], [[0, 128], [1, 1024]], 0))

            # HAM warm-up: ~4us of junk matmuls on the first-loaded tile so
            # the PE clock is at 8/8 before the projection phase starts
            wu_ps = ps.tile([128, 512], F32, tag="mm", bufs=8, name="wu_ps")
            for wi in range(10):
                nc.tensor.matmul(wu_ps[:], rt_sb[:, 0:128], rt_sb[:, 0:512],
                                 start=(wi == 0), stop=(wi == 9))

            # ---- projections: QT/KT [qdim part, seq], V [seq part, vdim] ----
            qt_sb = [projpool.tile([128, L], BF16, name=f"qt{m}") for m in range(4)]
            kt_sb = [projpool.tile([128, L], BF16, name=f"kt{m}") for m in range(4)]
            v_sb = [projpool.tile([128, 512], BF16, name=f"v{t}") for t in range(NT)]

            with tc.tile_pool(name="p1", bufs=1) as p1:
                xk = [p1.tile([128, L], F32R, name=f"xk{k}") for k in range(8)]
                wqk = [p1.tile([128, 512], F32R, name=f"wqk{k}") for k in range(8)]
                wkk = [p1.tile([128, 512], F32R, name=f"wkk{k}") for k in range(8)]
                wvk = [p1.tile([128, 512], F32R, name=f"wvk{k}") for k in range(8)]
                for k in range(8):
                    ksl = slice(k * 128, (k + 1) * 128)
                    nc.sync.dma_start(xk[k][:], xT[ksl, :])
                    nc.sync.dma_start(wqk[k][:], wq[ksl, :])
                    nc.sync.dma_start(wkk[k][:], wk[ksl, :])
                    nc.sync.dma_start(wvk[k][:], wv[ksl, :])

                for m in range(4):
                    msl = slice(m * 128, (m + 1) * 128)
                    for c in range(2):
                        csl = slice(c * 512, (c + 1) * 512)
                        qt_ps = ps.tile([128, 512], F32, tag="mm",
                                        bufs=8, name=f"qtp{m}{c}")
                        kt_ps = ps.tile([128, 512], F32, tag="mm",
                                        bufs=8, name=f"ktp{m}{c}")
                        for k in range(8):
                            nc.tensor.matmul(qt_ps[:], wqk[k][:, msl],
                                             xk[k][:, csl],
                                             start=(k == 0), stop=(k == 7))
                        for k in range(8):
                            nc.tensor.matmul(kt_ps[:], wkk[k][:, msl],
                                             xk[k][:, csl],
                                             start=(k == 0), stop=(k == 7))
                        nc.any.tensor_scalar_add(qt_sb[m][:, csl], qt_ps[:],
                                                 bq_sb[:, m:m + 1])
                        nc.any.tensor_scalar_add(kt_sb[m][:, csl], kt_ps[:],
                                                 bk_sb[:, m:m + 1])
                for t in range(NT):
                    tsl = slice(t * 128, (t + 1) * 128)
                    v_ps = ps.tile([128, 512], F32, tag="mm", bufs=8,
                                   name=f"vp{t}")
                    for k in range(8):
                        nc.tensor.matmul(v_ps[:], xk[k][:, tsl], wvk[k][:],
                                         start=(k == 0), stop=(k == 7))
                    nc.vector.tensor_tensor(
                        v_sb[t][:], v_ps[:], bv_sb[:], mybir.AluOpType.add)

            # ---- attention per head pair ----
            work = tc.alloc_tile_pool(name="work", bufs=3)
            apool = tc.alloc_tile_pool(name="attn", bufs=2)
            opool = tc.alloc_tile_pool(name="outp", bufs=3)
            ctxT_sb = [None] * NPAIR

            def emit_qe(p, t, h):
                qt_p = qt_sb[p]
                w0 = 896 - 128 * t
                hs = slice(64 * h, 64 * h + 64)
                isl = slice(t * 128, (t + 1) * 128)
                qe_sb = work.tile([128, 1152], BF16, tag="qe")
                for ci, (c0, cw) in enumerate(((0, 512), (512, 384),
                                               (896, 256))):
                    qe_ps = ps.tile([128, 512], F32, tag="mm",
                                    bufs=8, name=f"qe{p}{t}{h}{ci}")
                    nc.tensor.matmul(
                        qe_ps[:, :cw], qt_p[hs, isl],
                        rt_sb[hs, w0 + c0:w0 + c0 + cw],
                        start=True, stop=True)
                    nc.any.tensor_copy(qe_sb[:, c0:c0 + cw], qe_ps[:, :cw])
                # skew gather: bias[q, j] = qe_sb[q, 127 - q + j]
                bias_sb = work.tile([128, L], BF16, tag="bias", bufs=7,
                                    name="bias_sb")
                nc.sync.dma_start(
                    bias_sb[:],
                    _ap_with(qe_sb[:, 0:1024], [[1151, 128], [1, 1024]], 127))
                return bias_sb

            def emit_s(p, t, h, bias_sb, attn_h, sums_h):
                qt_p = qt_sb[p]
                kt_p = kt_sb[p]
                hs = slice(64 * h, 64 * h + 64)
                isl = slice(t * 128, (t + 1) * 128)
                s_sb = work.tile([128, L], F32, tag="ssb", bufs=2)
                for c in range(2):
                    csl = slice(c * 512, (c + 1) * 512)
                    s_ps = ps.tile([128, 512], F32, tag="mm", bufs=8,
                                   name=f"s{p}{t}{h}{c}")
                    nc.tensor.matmul(
                        s_ps[:], qt_p[hs, isl], kt_p[hs, csl],
                        start=True, stop=True)
                    nc.vector.tensor_tensor(
                        s_sb[:, csl], s_ps[:], bias_sb[:, csl],
                        mybir.AluOpType.add)
                nc.scalar.activation(
                    attn_h[h][:, t, :], s_sb[:],
                    mybir.ActivationFunctionType.Exp,
                    accum_out=sums_h[h][:, t:t + 1])
                recip = work.tile([128, 1], F32, tag="recip1", name="recip")
                nc.vector.reciprocal(recip[:], sums_h[h][:, t:t + 1])
                nc.any.tensor_scalar_mul(
                    attn_h[h][:, t, :], attn_h[h][:, t, :], recip[:])

            PFD = 4  # bias prefetch distance, in (t, h) iterations

            def attn_phase(p):
                """scores/bias/exp for pair p with the qe->skew chain emitted
                PFD iterations ahead of the consuming score block."""
                attn_h = [apool.tile([128, NT, L], BF16, name=f"attn{p}_{h}",
                                     tag="attn", bufs=3) for h in range(2)]
                sums_h = [work.tile([128, NT], F32, tag="sums", bufs=4,
                                    name=f"sums{p}_{h}") for h in range(2)]
                iters = [(t, h) for t in range(NT) for h in range(2)]
                bias_tiles = {}
                for idx in range(len(iters) + PFD):
                    if idx < len(iters):
                        t, h = iters[idx]
                        bias_tiles[idx] = emit_qe(p, t, h)
                    if idx >= PFD:
                        t, h = iters[idx - PFD]
                        emit_s(p, t, h, bias_tiles.pop(idx - PFD),
                               attn_h, sums_h)

                aT = [apool.tile([128, NT, NT, 128], BF16, tag="aT",
                                 name=f"aT{p}_{h}") for h in range(2)]
                for h in range(2):
                    nc.sync.dma_start(aT[h][:], attn_h[h][:], transpose=True)
                return (aT,)

            def av_phase(p, aT):
                ctxT_ps = [ps.tile([128, 512], F32, tag="mm", bufs=8,
                                   name=f"ctx{p}{c}") for c in range(2)]
                for jt in range(NT):
                    for c in range(2):
                        for h in range(2):
                            nc.tensor.matmul(
                                ctxT_ps[c][64 * h:64 * h + 64, :],
                                v_sb[jt][:, 64 * (2 * p + h):64 * (2 * p + h) + 64],
                                aT[h][:, 4 * c:4 * (c + 1), jt, :],
                                start=(jt == 0), stop=(jt == NT - 1))
                ctx = projpool.tile([128, L], F32R, name=f"ctxT{p}")
                for c in range(2):
                    nc.any.tensor_copy(ctx[:, c * 512:(c + 1) * 512],
                                       ctxT_ps[c][:])
                ctxT_sb[p] = ctx

            # software pipeline: attn(p) -> AV(p-1)
            prev = None
            for p in range(NPAIR):
                cur = attn_phase(p)
                if prev is not None:
                    av_phase(p - 1, *prev)
                prev = cur
            av_phase(NPAIR - 1, *prev)

            # ---- output projection (transpose-mode: ctx[i,hd] @ Wo[hd,o]) ----
            wo_sb = [wpool.tile([128, L], F32R, name=f"wo{m}") for m in range(4)]
            for m in range(4):
                nc.sync.dma_start(wo_sb[m][:], wo[m * 128:(m + 1) * 128, :])
            for t in range(NT):
                isl = slice(t * 128, (t + 1) * 128)
                for c in range(2):
                    o_ps = ps.tile([128, 512], F32, tag="mm", bufs=8,
                                   name=f"o{t}{c}")
                    for m in range(4):
                        nc.tensor.matmul(
                            o_ps[:], ctxT_sb[m][:, isl],
                            wo_sb[m][:, c * 512:(c + 1) * 512],
                            start=(m == 0), stop=(m == 3))
                    o_sb = opool.tile([128, 512], F32, tag="osb")
                    nc.vector.tensor_tensor(
                        o_sb[:], o_ps[:], bo_sb[:, c * 512:(c + 1) * 512],
                        mybir.AluOpType.add)
                    nc.sync.dma_start(out[isl, c * 512:(c + 1) * 512], o_sb[:])
            opool.release()
            apool.release()
            work.release()

    _split_multi_waits(nc)
    return nc


_cached = {}


def _get_program():
    if "nc" not in _cached:
        _cached["nc"] = _build_program()
    return _cached["nc"]


def kernel(x, Wq, bq, Wk, bk, Wv, bv, Wo, bo, rel_emb, _timing=None):
    x = np.asarray(x, np.float32)
    Wq = np.asarray(Wq, np.float32)
    Wk = np.asarray(Wk, np.float32)
    Wv = np.asarray(Wv, np.float32)
    Wo = np.asarray(Wo, np.float32)
    bq_ = np.asarray(bq, np.float32)
    bk_ = np.asarray(bk, np.float32)
    bv_ = np.asarray(bv, np.float32)
    bo_ = np.asarray(bo, np.float32)
    rel = np.asarray(rel_emb, np.float32)

    # flipped rel table, transposed, duplicated on both 64-partition halves,
    # padded to 2048 cols
    rt_half = rel[::-1, :].T  # [64, 2047]
    rt_np = np.zeros((128, 2048), ml_dtypes.bfloat16)
    rt_np[0:64, 0:2047] = rt_half.astype(ml_dtypes.bfloat16)
    rt_np[64:128, 0:2047] = rt_half.astype(ml_dtypes.bfloat16)

    in_maps = []
    for core in range(8):
        b, g = divmod(core, 2)
        cols = slice(g * 512, (g + 1) * 512)
        in_maps.append({
            "xT": np.ascontiguousarray(x[b].T),
            "wq": np.ascontiguousarray(Wq[:, cols]),
            "wk": np.ascontiguousarray(Wk[:, cols]) / 8.0,
            "wv": np.ascontiguousarray(Wv[:, cols]),
            "wo": np.ascontiguousarray(Wo[cols, :]),
            "rt": rt_np,
            "bq": np.ascontiguousarray(bq_[cols]),
            "bk": np.ascontiguousarray(bk_[cols]) / 8.0,
            "bv": np.ascontiguousarray(bv_[cols]),
            "bo": bo_ if g == 0 else np.zeros_like(bo_),
        })

    nc = _get_program()
    kwargs = {}
    if _timing is not None:
        kwargs = dict(trace=True, trace_cores=list(range(8)))
    r = run_bass_kernel_spmd(nc, in_maps, core_ids=list(range(8)), **kwargs)
    if _timing is not None:
        _timing["exec_time_ns"] = r.exec_time_ns
        _timing["mean_exec_time_ns"] = r.mean_exec_time_ns
        _timing["trace"] = r.instructions_and_trace
    outs = [r.results[c]["out"] for c in range(8)]
    return np.stack([outs[2 * b] + outs[2 * b + 1] for b in range(B)], axis=0)

